# revision 1
# baseline (speedup 1.0000x reference)
"""Max-dilated conv2d kernel for Trainium2 (Bass/Tile), 8-core data parallel.

out[b,oc,oh,ow] = max_{ic,kh,kw} x[b,ic,oh+2*kh, ow+2*kw] * w[oc,ic,kh,kw]

Shapes (hardcoded): x (8,32,68,68) f32, w (32,32,3,3) f32, out (8,32,64,64) f32.
stride=1, dilation=2.

Sharding: batch across the 8 NeuronCores (1 image per core), weights replicated.

Per-core algorithm:
  Partition layout p = icq*32 + oc  (icq in 0..3, oc in 0..31).
  The 32 input channels are processed in 8 groups of 4 (ic = h*4 + icq).
  For each group h, x[ic] rows are broadcast 32x across partitions (DMA from
  DRAM with a stride-0 access pattern) so partition p holds x[h*4 + p//32].
  For each (kh,kw) one fused VectorE op (scalar_tensor_tensor) does
      acc[p] = max(acc[p], x_shifted[p] * wv[p])
  where wv[p] = w[p%32, h*4 + p//32, kh, kw] is a per-partition scalar and the
  shifted window is a plain slice [2kh:2kh+64, 2kw:2kw+64] of the 68x68 image.
  After all 72 planes, a 2-level cross-partition tree-max (128->64->32)
  reduces the 4 icq slots, leaving out[oc] on partitions 0..31.  TensorTensor
  needs equal base partitions for SBUF operands, so each level realigns the
  upper partition half with SBUF->SBUF DMAs first.

mode="mixed" additionally offloads most planes to ScalarE as fp16 products
max-accumulated by VectorE tensor_tensor at its 2x fp16 rate (~5e-4 rel err).
mode="fp32" (default) is bit-exact.
"""

import sys

sys.path.insert(0, "/opt/trn_rl_repo")

import numpy as np

import concourse.bacc as bacc
import concourse.tile as tile
from concourse import mybir
from concourse import bass_utils

IC, OC, K = 32, 32, 3
H = W = 68
OH = OW = 64
DH = DW = 2
NCORES = 8
NGROUPS = 8  # ic groups of 4
PLANES = NGROUPS * K * K  # 72
F32 = mybir.dt.float32
F16 = mybir.dt.float16

MODE = "fp32"
# mixed mode: how many of the 9 planes per group stay on the exact fp32
# fused-stt path (the rest go ScalarE-fp16-product + VectorE fp16 max)
STT_PER_GROUP = [3, 2, 3, 2, 3, 2, 3, 2]

_cache: dict = {}


def _build(mode: str = MODE):
    """Build + compile the per-core Bass program (same NEFF on all cores)."""
    if mode in _cache:
        return _cache[mode]

    nc = bacc.Bacc("TRN2", debug=False, num_devices=NCORES)
    x_d = nc.dram_tensor("x", [IC, H, W], F32, kind="ExternalInput").ap()
    wv_d = nc.dram_tensor("wv", [128, PLANES], F32, kind="ExternalInput").ap()
    out_d = nc.dram_tensor("out", [OC, OH, OW], F32, kind="ExternalOutput").ap()

    mult = mybir.AluOpType.mult
    amax = mybir.AluOpType.max

    with tile.TileContext(nc) as tc:
        with (
            tc.tile_pool(name="const", bufs=1) as cpool,
            tc.tile_pool(name="xrep", bufs=4) as xpool,
            tc.tile_pool(name="work", bufs=1) as wpool,
        ):
            wv_sb = cpool.tile([128, PLANES], F32, tag="wv")
            nc.sync.dma_start(wv_sb[:, :], wv_d[:, :])

            acc_v = wpool.tile([128, OH, OW], F32, tag="acc_v")
            acc_h = (
                wpool.tile([128, OH, OW], F16, tag="acc_h", name="acc_h")
                if mode == "mixed"
                else None
            )

            # in mixed mode ScalarE is saturated with products — keep it off
            # DMA dispatch duty there
            dma_engines = (
                [nc.sync, nc.scalar, nc.gpsimd]
                if mode == "fp32"
                else [nc.sync, nc.gpsimd]
            )
            first_v = True
            first_h = True
            ei = 0
            for h in range(NGROUPS):
                xr = xpool.tile([128, H, W], F32, tag="xr")
                if h == 0:
                    # startup: all 3 DMA-capable dispatch engines, finer
                    # chunks so the first plane can start on partial data
                    for s in range(4):
                        r0, r1 = s * 17, (s + 1) * 17
                        for icq in range(4):
                            src = (
                                x_d[h * 4 + icq]
                                .unsqueeze(0)
                                .broadcast_to([32, H, W])
                            )
                            dma_engines[ei % len(dma_engines)].dma_start(
                                xr[icq * 32 : (icq + 1) * 32, r0:r1],
                                src[:, r0:r1],
                            )
                            ei += 1
                else:
                    for icq in range(4):
                        src = (
                            x_d[h * 4 + icq].unsqueeze(0).broadcast_to([32, H, W])
                        )
                        for s in range(2):
                            r0, r1 = s * 34, (s + 1) * 34
                            dma_engines[ei % len(dma_engines)].dma_start(
                                xr[icq * 32 : (icq + 1) * 32, r0:r1],
                                src[:, r0:r1],
                            )
                            ei += 1

                n_stt = K * K if mode == "fp32" else STT_PER_GROUP[h]
                last = h == NGROUPS - 1
                # last group pixel-split so the reduction tree's DMAs overlap
                # remaining compute; first plane row-split so compute starts
                # on partial group-0 data
                if last:
                    splits = [(0, 32), (32, 64)]
                elif h == 0:
                    splits = None  # per-plane special-case below
                else:
                    splits = [(0, 64)]

                for k in range(K * K):
                    kh, kw = divmod(k, K)
                    j = h * (K * K) + k
                    wcol = wv_sb[:, j : j + 1]
                    on_stt = k >= K * K - n_stt
                    if h == 0:
                        # consume group-0 rows as the 17-row DMA chunks land
                        if k < 3:
                            ksplits = [(0, 13), (13, 30), (30, 47), (47, 64)]
                        elif k < 6:
                            ksplits = [(0, 30), (30, 64)]
                        else:
                            ksplits = [(0, 64)]
                    else:
                        ksplits = splits
                    for a, b in ksplits:
                        view = xr[
                            :, DH * kh + a : DH * kh + b, DW * kw : DW * kw + OW
                        ]
                        if on_stt:
                            accw = acc_v[:, a:b, :]
                            if first_v:
                                nc.vector.tensor_scalar_mul(accw, view, wcol)
                            else:
                                nc.vector.scalar_tensor_tensor(
                                    accw, view, wcol, accw, mult, amax
                                )
                        else:
                            acch = acc_h[:, a:b, :]
                            if first_h:
                                nc.scalar.mul(acch, view, wcol)
                            else:
                                prod = xpool.tile(
                                    [128, b - a, OW], F16, tag="prod", name="prod", bufs=6
                                )
                                nc.scalar.mul(prod[:], view, wcol)
                                nc.vector.tensor_max(acch, acch, prod[:])
                    if on_stt:
                        first_v = False
                    else:
                        first_h = False

            # Cross-partition tree-max in two pixel halves so half A's DMAs
            # overlap half B's compute.
            t64 = wpool.tile([64, OH, OW], F32, tag="t64")
            out_sb = wpool.tile([32, OH, OW], F32, tag="out_sb")
            for hi, (a, b) in enumerate([(0, 32), (32, 64)]):
                if mode == "mixed":
                    nc.vector.tensor_max(
                        acc_v[:, a:b, :], acc_v[:, a:b, :], acc_h[:, a:b, :]
                    )
                for s in range(2):
                    r0 = a + s * 16
                    r1 = r0 + 16
                    dma_engines[(hi + s) % len(dma_engines)].dma_start(
                        t64[:, r0:r1, :], acc_v[64:128, r0:r1, :]
                    )
                nc.vector.tensor_max(
                    t64[:, a:b, :], t64[:, a:b, :], acc_v[0:64, a:b, :]
                )
                dma_engines[hi % len(dma_engines)].dma_start(
                    out_sb[:, a:b, :], t64[32:64, a:b, :]
                )
                nc.vector.tensor_max(
                    out_sb[:, a:b, :], out_sb[:, a:b, :], t64[0:32, a:b, :]
                )
                for s in range(2):
                    r0 = a + s * 16
                    r1 = r0 + 16
                    dma_engines[(hi + s) % len(dma_engines)].dma_start(
                        out_d[:, r0:r1, :], out_sb[:, r0:r1, :]
                    )

    nc.compile()
    _cache[mode] = nc
    return nc


def _make_wv(w: np.ndarray) -> np.ndarray:
    """wv[p, h*9+k] = w[p%32, h*4 + p//32, kh, kw] with k = kh*3+kw."""
    wr = w.reshape(OC, NGROUPS, 4, K * K)  # (oc, h, icq, k); ic = h*4+icq
    wv = wr.transpose(2, 0, 1, 3).reshape(4 * OC, PLANES)  # (icq*32+oc, h*9+k)
    return np.ascontiguousarray(wv, dtype=np.float32)


def _ensure_axon_hooks_module():
    """bass_utils imports antenv.axon_hooks when tracing is requested (e.g.
    via BASS_TRACE).  The module is absent on this image; provide a stub so
    the run degrades to untraced instead of crashing."""
    try:
        import antenv.axon_hooks  # noqa: F401
    except Exception:
        import types

        mod = types.ModuleType("antenv.axon_hooks")
        mod._hook = None
        mod.get_axon_ntff_profile_hook = lambda: getattr(mod, "_hook", None)
        mod.set_axon_ntff_profile_hook = lambda h: setattr(mod, "_hook", h)
        sys.modules["antenv.axon_hooks"] = mod
        try:
            import antenv

            antenv.axon_hooks = mod
        except Exception:
            pass


def kernel(x, weight, stride_h=1, stride_w=1, dilation_h=2, dilation_w=2):
    _ensure_axon_hooks_module()
    x = np.ascontiguousarray(np.asarray(x, dtype=np.float32))
    w = np.ascontiguousarray(np.asarray(weight, dtype=np.float32))
    assert int(stride_h) == 1 and int(stride_w) == 1
    assert int(dilation_h) == DH and int(dilation_w) == DW
    B = x.shape[0]
    assert x.shape == (B, IC, H, W) and w.shape == (OC, IC, K, K)
    assert B == NCORES

    wv = _make_wv(w)
    nc = _build(MODE)
    in_maps = [{"x": x[b], "wv": wv} for b in range(B)]
    res = bass_utils.run_bass_kernel_spmd(nc, in_maps, core_ids=list(range(B)))
    out = np.stack([r["out"] for r in res.results], axis=0)
    return out.astype(np.float32)


def run_traced(x, weight, mode=MODE, **trace_kwargs):
    """Like kernel() but with hardware profiling; returns (out, BassKernelResults)."""
    x = np.ascontiguousarray(np.asarray(x, dtype=np.float32))
    w = np.ascontiguousarray(np.asarray(weight, dtype=np.float32))
    wv = _make_wv(w)
    nc = _build(mode)
    in_maps = [{"x": x[b], "wv": wv} for b in range(x.shape[0])]
    res = bass_utils.run_bass_kernel_spmd(
        nc, in_maps, core_ids=list(range(x.shape[0])), trace=True, **trace_kwargs
    )
    out = np.stack([r["out"] for r in res.results], axis=0)
    return out.astype(np.float32), res

